# revision 11
# baseline (speedup 1.0000x reference)
"""MinGRU (2-layer, residual) Trainium2 Bass kernel.

Problem: B=8, S=2048, D=H=1024, L=2.
Sharding: data-parallel over batch across 8 NeuronCores (1 sample/core);
weights replicated. All tensors device-side are channel-major (channels on
partitions, sequence on the free dimension) so no transposes are needed on
device; the host transposes x / W once and transposes the output back.

Per-core pipeline (per layer):
  gh^T = W^T-blocks @ x^T            TensorE, fp32r (full rate, ~1e-4 rel err)
  c = sigmoid(-(gate+b))             ScalarE from PSUM, fused bias/scale
  z = sigmoid(gate+b)                ScalarE
  g = max(hidden+b+0.5, sigmoid(hidden+b))   ScalarE + fused DVE scalar_tensor_tensor
  u = z*g                            DVE
  h_t = c_t*h_{t-1} + u_t            DVE tensor_tensor_scan along S
  residual adds                      GPSIMD

Layer 0 is t-outer: x streams in 512-token chunks, scan chains across chunks
via a small carry buffer; h+x lands in the resident inp buffer (fp32r), which
is layer 1's matmul rhs. Layer 1 is e-outer: weights stream per 128-channel
block (never fully resident -> no layer-boundary weight-load stall), one
full-sequence scan per row, output row DMA'd back as one contiguous 1MB
transfer.
"""
import numpy as np

import concourse.bass as bass
import concourse.mybir as mybir
import concourse.tile as tile
from concourse import bacc
from concourse.bass_utils import run_bass_kernel_spmd

F32 = mybir.dt.float32
F32R = mybir.dt.float32r
SIG = mybir.ActivationFunctionType.Sigmoid
MULT = mybir.AluOpType.mult
ADD = mybir.AluOpType.add
MAX = mybir.AluOpType.max

B, S, D, H = 8, 2048, 1024, 1024
KT = D // 128          # 8  k-tiles (contraction)
HT = H // 128          # 8  h-tiles (per gate/hidden half)
ET = 2 * H // 128      # 16 e-tiles (2H output channels)
TC = 4                 # t-chunks
CS = S // TC           # 512 chunk size (PSUM bank width in fp32)

_CACHED = {}


def build():
    nc = bacc.Bacc(dynamic_dma_scratch_size=2048)

    xt = nc.dram_tensor("xt", [KT, 128, S], F32R, kind="ExternalInput")
    w0t = nc.dram_tensor("w0t", [KT, 128, 2 * H], F32R, kind="ExternalInput")
    w1t = nc.dram_tensor("w1t", [KT, 128, 2 * H], F32R, kind="ExternalInput")
    bias = nc.dram_tensor("bias", [128, 2 * ET], F32, kind="ExternalInput")
    aux = nc.dram_tensor("aux", [128, 2 * ET], F32, kind="ExternalInput")
    hinit = nc.dram_tensor("hinit", [128, 2 * HT], F32, kind="ExternalInput")

    outT = nc.dram_tensor("outT", [HT, 128, S], F32, kind="ExternalOutput")
    hfin = nc.dram_tensor("hfin", [2, HT, 128], F32, kind="ExternalOutput")

    with tile.TileContext(nc) as tc:
        with (
            tc.tile_pool(name="singles", bufs=1) as singles,
            tc.tile_pool(name="inp", bufs=1) as inppool,
            tc.tile_pool(name="psum", bufs=8, space="PSUM") as psum_pool,
        ):
            bias_sb = singles.tile([128, 2 * ET], F32)
            aux_sb = singles.tile([128, 2 * ET], F32)
            hinit_sb = singles.tile([128, 2 * HT], F32)
            hst0 = singles.tile([128, HT], F32)
            hst1 = singles.tile([128, HT], F32)
            nc.sync.dma_start(out=bias_sb, in_=bias.ap())
            nc.sync.dma_start(out=aux_sb, in_=aux.ap())
            nc.sync.dma_start(out=hinit_sb, in_=hinit.ap())

            inp_sb = inppool.tile([128, KT, S], F32R, tag="inp")

            # ================= layer 0 (t-outer, streamed x) =================
            with (
                tc.tile_pool(name="wpool", bufs=1) as wpool,
                tc.tile_pool(name="chunks", bufs=2) as chunks,
                tc.tile_pool(name="cz", bufs=1) as czpool,
            ):
                # first x chunk loads BEFORE the weights so the first psum
                # group isn't gated on the full 8MB weight load
                x0 = chunks.tile([128, KT, CS], F32R, tag="chunkbuf")
                for k in range(KT):
                    nc.sync.dma_start(out=x0[:, k, :], in_=xt.ap()[k, :, 0:CS])

                w_sb = wpool.tile([128, KT, 2 * H], F32R, tag="w")
                # e-sliced weight loads in consumption order (gate i, hidden
                # i+HT pairs): psum group i only depends on its own slices
                for i in range(HT):
                    for e in (i, HT + i):
                        for k in range(KT):
                            nc.sync.dma_start(
                                out=w_sb[:, k, e * 128:(e + 1) * 128],
                                in_=w0t.ap()[k, :, e * 128:(e + 1) * 128],
                            )

                for t in range(TC):
                    ts0, ts1 = t * CS, (t + 1) * CS
                    if t == 0:
                        x_t = x0
                    else:
                        x_t = chunks.tile([128, KT, CS], F32R, tag="chunkbuf")
                        for k in range(KT):
                            nc.sync.dma_start(
                                out=x_t[:, k, :], in_=xt.ap()[k, :, ts0:ts1]
                            )

                    c_t = czpool.tile([128, HT, CS], F32, tag="c")
                    z_t = czpool.tile([128, HT, CS], F32, tag="z")
                    gu_t = czpool.tile([128, HT, CS], F32, tag="gu")

                    for i in range(HT):
                        # ---- gate e-tile i ----
                        ps = psum_pool.tile([128, CS], F32, tag="ps")
                        for k in range(KT):
                            nc.tensor.matmul(
                                ps,
                                lhsT=w_sb[:, k, i * 128:(i + 1) * 128],
                                rhs=x_t[:, k, :],
                                start=(k == 0),
                                stop=(k == KT - 1),
                            )
                        nc.scalar.activation(
                            out=c_t[:, i, :], in_=ps, func=SIG,
                            bias=aux_sb[:, i:i + 1], scale=-1.0,
                        )
                        nc.scalar.activation(
                            out=z_t[:, i, :], in_=ps, func=SIG,
                            bias=bias_sb[:, i:i + 1],
                        )
                        # ---- hidden e-tile i+HT ----
                        ph = psum_pool.tile([128, CS], F32, tag="ps")
                        e = HT + i
                        for k in range(KT):
                            nc.tensor.matmul(
                                ph,
                                lhsT=w_sb[:, k, e * 128:(e + 1) * 128],
                                rhs=x_t[:, k, :],
                                start=(k == 0),
                                stop=(k == KT - 1),
                            )
                        nc.scalar.activation(
                            out=gu_t[:, i, :], in_=ph, func=SIG,
                            bias=bias_sb[:, e:e + 1],
                        )
                        # g = (hidden + (b+0.5)) max sigmoid(hidden+b)
                        nc.vector.scalar_tensor_tensor(
                            out=gu_t[:, i, :], in0=ph,
                            scalar=aux_sb[:, e:e + 1],
                            in1=gu_t[:, i, :], op0=ADD, op1=MAX,
                        )
                        # u = z * g (on GPSIMD: keeps DVE free for scans)
                        nc.gpsimd.tensor_mul(gu_t[:, i, :], z_t[:, i, :], gu_t[:, i, :])
                        # ---- scan (chained across chunks via hst0) ----
                        init = (hinit_sb[:, i:i + 1] if t == 0
                                else hst0[:, i:i + 1])
                        nc.vector.tensor_tensor_scan(
                            out=inp_sb[:, i, ts0:ts1],
                            data0=c_t[:, i, :], data1=gu_t[:, i, :],
                            initial=init, op0=MULT, op1=ADD,
                        )
                        # carry pre-residual h for next chunk's initial
                        nc.gpsimd.tensor_copy(
                            hst0[:, i:i + 1], inp_sb[:, i, ts1 - 1:ts1].bitcast(F32)
                        )

                    # residual add, immediately (scan chain carries via hst0)
                    nc.gpsimd.tensor_add(
                        inp_sb[:, :, ts0:ts1], inp_sb[:, :, ts0:ts1], x_t
                    )

                # h0_final = pre-residual h at s = S-1 = the last carry
                for i in range(HT):
                    nc.sync.dma_start(
                        out=hfin.ap()[0, i].unsqueeze(1),
                        in_=hst0[:, i:i + 1],
                    )

            # ============ layer 1 (e-outer, streamed w1, resident rhs) =======
            with (
                tc.tile_pool(name="w1pool", bufs=4) as w1pool,
                tc.tile_pool(name="rows", bufs=2) as rows,
            ):
                for i in range(HT):
                    wg = w1pool.tile([128, KT, 128], F32R, tag="w1e")
                    wh = w1pool.tile([128, KT, 128], F32R, tag="w1e")
                    for k in range(KT):
                        nc.sync.dma_start(
                            out=wg[:, k, :],
                            in_=w1t.ap()[k, :, i * 128:(i + 1) * 128],
                        )
                    for k in range(KT):
                        nc.sync.dma_start(
                            out=wh[:, k, :],
                            in_=w1t.ap()[k, :, (HT + i) * 128:(HT + i + 1) * 128],
                        )

                    c_r = rows.tile([128, S], F32, tag="c1")
                    z_r = rows.tile([128, S], F32, tag="z1")
                    gu_r = rows.tile([128, S], F32, tag="gu1")

                    for t in range(TC):
                        ts0, ts1 = t * CS, (t + 1) * CS
                        ps = psum_pool.tile([128, CS], F32, tag="ps")
                        for k in range(KT):
                            nc.tensor.matmul(
                                ps, lhsT=wg[:, k, :],
                                rhs=inp_sb[:, k, ts0:ts1],
                                start=(k == 0), stop=(k == KT - 1),
                            )
                        nc.scalar.activation(
                            out=c_r[:, ts0:ts1], in_=ps, func=SIG,
                            bias=aux_sb[:, ET + i:ET + i + 1], scale=-1.0,
                        )
                        nc.scalar.activation(
                            out=z_r[:, ts0:ts1], in_=ps, func=SIG,
                            bias=bias_sb[:, ET + i:ET + i + 1],
                        )
                    e = ET + HT + i
                    for t in range(TC):
                        ts0, ts1 = t * CS, (t + 1) * CS
                        ph = psum_pool.tile([128, CS], F32, tag="ps")
                        for k in range(KT):
                            nc.tensor.matmul(
                                ph, lhsT=wh[:, k, :],
                                rhs=inp_sb[:, k, ts0:ts1],
                                start=(k == 0), stop=(k == KT - 1),
                            )
                        nc.scalar.activation(
                            out=gu_r[:, ts0:ts1], in_=ph, func=SIG,
                            bias=bias_sb[:, e:e + 1],
                        )
                        nc.vector.scalar_tensor_tensor(
                            out=gu_r[:, ts0:ts1], in0=ph,
                            scalar=aux_sb[:, e:e + 1],
                            in1=gu_r[:, ts0:ts1], op0=ADD, op1=MAX,
                        )
                    # u / scan / residual / DMA, chunk-pipelined so the
                    # kernel tail after the final matmul stays short
                    h1_r = rows.tile([128, S], F32, tag="h1")
                    for t in range(TC):
                        ts0, ts1 = t * CS, (t + 1) * CS
                        nc.gpsimd.tensor_mul(
                            gu_r[:, ts0:ts1], z_r[:, ts0:ts1], gu_r[:, ts0:ts1]
                        )
                        init = (hinit_sb[:, HT + i:HT + i + 1] if t == 0
                                else hst1[:, i:i + 1])
                        nc.vector.tensor_tensor_scan(
                            out=h1_r[:, ts0:ts1], data0=c_r[:, ts0:ts1],
                            data1=gu_r[:, ts0:ts1],
                            initial=init, op0=MULT, op1=ADD,
                        )
                        if t < TC - 1:
                            nc.gpsimd.tensor_copy(
                                hst1[:, i:i + 1], h1_r[:, ts1 - 1:ts1]
                            )
                        else:
                            nc.sync.dma_start(
                                out=hfin.ap()[1, i].unsqueeze(1),
                                in_=h1_r[:, S - 1:S],
                            )
                        nc.gpsimd.tensor_add(
                            h1_r[:, ts0:ts1], h1_r[:, ts0:ts1],
                            inp_sb[:, i, ts0:ts1].bitcast(F32),
                        )
                        nc.sync.dma_start(
                            out=outT.ap()[i, :, ts0:ts1], in_=h1_r[:, ts0:ts1]
                        )

    nc.compile()
    return nc


def _prepare_shared(w0, b0, w1, b1, h):
    w0t = np.ascontiguousarray(w0.T).reshape(KT, 128, 2 * H)
    w1t = np.ascontiguousarray(w1.T).reshape(KT, 128, 2 * H)
    # bias[:, l*16 + e] = b_l[e*128 : (e+1)*128]
    bias = np.concatenate(
        [b0.reshape(ET, 128).T, b1.reshape(ET, 128).T], axis=1
    ).astype(np.float32)
    aux0 = np.concatenate([-b0[:H].reshape(HT, 128).T,
                           b0[H:].reshape(HT, 128).T + 0.5], axis=1)
    aux1 = np.concatenate([-b1[:H].reshape(HT, 128).T,
                           b1[H:].reshape(HT, 128).T + 0.5], axis=1)
    aux = np.concatenate([aux0, aux1], axis=1).astype(np.float32)
    return w0t, w1t, bias, np.ascontiguousarray(aux)


def kernel(x, h, w0, b0, w1, b1):
    x = np.asarray(x, np.float32)
    h = np.asarray(h, np.float32)
    w0 = np.asarray(w0, np.float32)
    b0 = np.asarray(b0, np.float32)
    w1 = np.asarray(w1, np.float32)
    b1 = np.asarray(b1, np.float32)

    if "nc" not in _CACHED:
        _CACHED["nc"] = build()
    nc = _CACHED["nc"]

    w0t, w1t, bias, aux = _prepare_shared(w0, b0, w1, b1, h)
    in_maps = []
    for b in range(B):
        xt = np.ascontiguousarray(x[b].T).reshape(KT, 128, S)
        hinit = np.concatenate(
            [h[0, b, 0].reshape(HT, 128).T, h[1, b, 0].reshape(HT, 128).T],
            axis=1,
        ).astype(np.float32)
        in_maps.append({
            "xt": xt, "w0t": w0t, "w1t": w1t,
            "bias": bias, "aux": aux,
            "hinit": np.ascontiguousarray(hinit),
        })

    res = run_bass_kernel_spmd(nc, in_maps, core_ids=list(range(B)))

    out = np.empty((B, S, H), np.float32)
    hfinal = np.empty((2, B, 1, H), np.float32)
    for b in range(B):
        r = res.results[b]
        out[b] = r["outT"].reshape(H, S).T
        hfinal[:, b, 0, :] = r["hfin"].reshape(2, H)
    return out, hfinal


# revision 13
# speedup vs baseline: 1.1339x; 1.1339x over previous
"""MinGRU (2-layer, residual) Trainium2 Bass kernel.

Problem: B=8, S=2048, D=H=1024, L=2.
Sharding: data-parallel over batch across 8 NeuronCores (1 sample/core);
weights replicated. All tensors device-side are channel-major (channels on
partitions, sequence on the free dimension) so no transposes are needed on
device; the host transposes x / W once and transposes the output back. All
DRAM layouts are pre-tiled host-side so every DMA is fully contiguous.

Per-core pipeline (per layer):
  gh^T = W^T-blocks @ x^T            TensorE, fp32r (full rate, ~1e-4 rel err)
  c = sigmoid(-(gate+b))             ScalarE from PSUM, fused bias/scale
  z = sigmoid(gate+b)                ScalarE
  g = max(hidden+b+0.5, sigmoid(hidden+b))   ScalarE + fused DVE scalar_tensor_tensor
  u = z*g                            DVE
  h_t = c_t*h_{t-1} + u_t            DVE tensor_tensor_scan along S
  residual adds                      GPSIMD

Layer 0 is t-outer: x streams in 512-token chunks, scan chains across chunks
via a small carry buffer; h+x lands in the resident inp buffer (fp32r), which
is layer 1's matmul rhs. Layer 1 is e-outer with gate/hidden interleaved per
chunk: weights stream per 128-channel block (prefetched during layer 0), and
each row's scan chain overlaps its own matmuls, keeping the kernel tail short.
"""
import numpy as np

import concourse.bass as bass
import concourse.mybir as mybir
import concourse.tile as tile
from concourse import bacc
from concourse.bass_utils import run_bass_kernel_spmd

F32 = mybir.dt.float32
F32R = mybir.dt.float32r
SIG = mybir.ActivationFunctionType.Sigmoid
MULT = mybir.AluOpType.mult
ADD = mybir.AluOpType.add
MAX = mybir.AluOpType.max

B, S, D, H = 8, 2048, 1024, 1024
KT = D // 128          # 8  k-tiles (contraction)
HT = H // 128          # 8  h-tiles (per gate/hidden half)
ET = 2 * H // 128      # 16 e-tiles (2H output channels)
TC = 4                 # t-chunks
CS = S // TC           # 512 chunk size (PSUM bank width in fp32)

_CACHED = {}


def build():
    nc = bacc.Bacc(dynamic_dma_scratch_size=2048)

    # all DRAM tensors pre-tiled so each DMA transfer is contiguous
    xt = nc.dram_tensor("xt", [TC, KT, 128, CS], F32R, kind="ExternalInput")
    w0t = nc.dram_tensor("w0t", [ET, KT, 128, 128], F32R, kind="ExternalInput")
    w1t = nc.dram_tensor("w1t", [ET, KT, 128, 128], F32R, kind="ExternalInput")
    bias = nc.dram_tensor("bias", [128, 2 * ET], F32, kind="ExternalInput")
    aux = nc.dram_tensor("aux", [128, 2 * ET], F32, kind="ExternalInput")
    hinit = nc.dram_tensor("hinit", [128, 2 * HT], F32, kind="ExternalInput")

    outT = nc.dram_tensor("outT", [HT, TC, 128, CS], F32, kind="ExternalOutput")
    hfin = nc.dram_tensor("hfin", [2, HT, 128], F32, kind="ExternalOutput")

    with tile.TileContext(nc) as tc:
        with (
            tc.tile_pool(name="singles", bufs=1) as singles,
            tc.tile_pool(name="inp", bufs=1) as inppool,
            tc.tile_pool(name="w1pool", bufs=3) as w1pool,
            tc.tile_pool(name="psum", bufs=8, space="PSUM") as psum_pool,
        ):
            bias_sb = singles.tile([128, 2 * ET], F32)
            aux_sb = singles.tile([128, 2 * ET], F32)
            hinit_sb = singles.tile([128, 2 * HT], F32)
            hst0 = singles.tile([128, HT], F32)
            hst1 = singles.tile([128, HT], F32)
            nc.sync.dma_start(out=bias_sb, in_=bias.ap())
            nc.sync.dma_start(out=aux_sb, in_=aux.ap())
            nc.sync.dma_start(out=hinit_sb, in_=hinit.ap())

            inp_sb = inppool.tile([128, KT, S], F32R, tag="inp")

            # ================= layer 0 (t-outer, streamed x) =================
            with (
                tc.tile_pool(name="wpool", bufs=1) as wpool,
                tc.tile_pool(name="chunks", bufs=2) as chunks,
                tc.tile_pool(name="cz", bufs=1) as czpool,
            ):
                # first x chunk loads BEFORE the weights so the first psum
                # group isn't gated on the full 8MB weight load
                x0 = chunks.tile([128, KT, CS], F32R, tag="chunkbuf")
                for k in range(KT):
                    nc.sync.dma_start(out=x0[:, k, :], in_=xt.ap()[0, k])

                w_sb = wpool.tile([128, KT, 2 * H], F32R, tag="w")
                # per-block weight loads in consumption order (gate i,
                # hidden i+HT pairs); each transfer is a contiguous 64KB
                for i in range(HT):
                    for e in (i, HT + i):
                        for k in range(KT):
                            nc.sync.dma_start(
                                out=w_sb[:, k, e * 128:(e + 1) * 128],
                                in_=w0t.ap()[e, k],
                            )

                for t in range(TC):
                    ts0, ts1 = t * CS, (t + 1) * CS
                    if t == 0:
                        x_t = x0
                    else:
                        x_t = chunks.tile([128, KT, CS], F32R, tag="chunkbuf")
                        for k in range(KT):
                            nc.sync.dma_start(out=x_t[:, k, :], in_=xt.ap()[t, k])

                    c_t = czpool.tile([128, HT, CS], F32, tag="c")
                    z_t = czpool.tile([128, HT, CS], F32, tag="z")
                    gu_t = czpool.tile([128, HT, CS], F32, tag="gu")

                    for i in range(HT):
                        # ---- gate e-tile i ----
                        ps = psum_pool.tile([128, CS], F32, tag="ps")
                        for k in range(KT):
                            nc.tensor.matmul(
                                ps,
                                lhsT=w_sb[:, k, i * 128:(i + 1) * 128],
                                rhs=x_t[:, k, :],
                                start=(k == 0),
                                stop=(k == KT - 1),
                            )
                        nc.scalar.activation(
                            out=c_t[:, i, :], in_=ps, func=SIG,
                            bias=aux_sb[:, i:i + 1], scale=-1.0,
                        )
                        nc.scalar.activation(
                            out=z_t[:, i, :], in_=ps, func=SIG,
                            bias=bias_sb[:, i:i + 1],
                        )
                        # ---- hidden e-tile i+HT ----
                        ph = psum_pool.tile([128, CS], F32, tag="ps")
                        e = HT + i
                        for k in range(KT):
                            nc.tensor.matmul(
                                ph,
                                lhsT=w_sb[:, k, e * 128:(e + 1) * 128],
                                rhs=x_t[:, k, :],
                                start=(k == 0),
                                stop=(k == KT - 1),
                            )
                        nc.scalar.activation(
                            out=gu_t[:, i, :], in_=ph, func=SIG,
                            bias=bias_sb[:, e:e + 1],
                        )
                        # g = (hidden + (b+0.5)) max sigmoid(hidden+b)
                        nc.vector.scalar_tensor_tensor(
                            out=gu_t[:, i, :], in0=ph,
                            scalar=aux_sb[:, e:e + 1],
                            in1=gu_t[:, i, :], op0=ADD, op1=MAX,
                        )
                        # u = z * g
                        nc.vector.tensor_mul(gu_t[:, i, :], z_t[:, i, :], gu_t[:, i, :])
                        # ---- scan (chained across chunks via hst0) ----
                        init = (hinit_sb[:, i:i + 1] if t == 0
                                else hst0[:, i:i + 1])
                        nc.vector.tensor_tensor_scan(
                            out=inp_sb[:, i, ts0:ts1],
                            data0=c_t[:, i, :], data1=gu_t[:, i, :],
                            initial=init, op0=MULT, op1=ADD,
                        )
                        # carry pre-residual h for next chunk's initial
                        nc.gpsimd.tensor_copy(
                            hst0[:, i:i + 1], inp_sb[:, i, ts1 - 1:ts1].bitcast(F32)
                        )

                    # residual add, immediately (scan chain carries via hst0)
                    nc.gpsimd.tensor_add(
                        inp_sb[:, :, ts0:ts1], inp_sb[:, :, ts0:ts1], x_t
                    )

                # h0_final = pre-residual h at s = S-1 = the last carry
                for i in range(HT):
                    nc.sync.dma_start(
                        out=hfin.ap()[0, i].unsqueeze(1),
                        in_=hst0[:, i:i + 1],
                    )

            # ==== layer 1 (e-outer, gate/hidden interleaved per chunk) ======
            with tc.tile_pool(name="rows", bufs=2) as rows:
                for i in range(HT):
                    wg = w1pool.tile([128, KT, 128], F32R, tag="w1e")
                    wh = w1pool.tile([128, KT, 128], F32R, tag="w1e")
                    for k in range(KT):
                        nc.sync.dma_start(out=wg[:, k, :], in_=w1t.ap()[i, k])
                    for k in range(KT):
                        nc.sync.dma_start(out=wh[:, k, :], in_=w1t.ap()[HT + i, k])

                    c_r = rows.tile([128, S], F32, tag="c1")
                    z_r = rows.tile([128, S], F32, tag="z1")
                    gu_r = rows.tile([128, S], F32, tag="gu1")
                    h1_r = rows.tile([128, S], F32, tag="h1")
                    e = ET + HT + i

                    for t in range(TC):
                        ts0, ts1 = t * CS, (t + 1) * CS
                        ps = psum_pool.tile([128, CS], F32, tag="ps")
                        for k in range(KT):
                            nc.tensor.matmul(
                                ps, lhsT=wg[:, k, :],
                                rhs=inp_sb[:, k, ts0:ts1],
                                start=(k == 0), stop=(k == KT - 1),
                            )
                        nc.scalar.activation(
                            out=c_r[:, ts0:ts1], in_=ps, func=SIG,
                            bias=aux_sb[:, ET + i:ET + i + 1], scale=-1.0,
                        )
                        nc.scalar.activation(
                            out=z_r[:, ts0:ts1], in_=ps, func=SIG,
                            bias=bias_sb[:, ET + i:ET + i + 1],
                        )
                        ph = psum_pool.tile([128, CS], F32, tag="ps")
                        for k in range(KT):
                            nc.tensor.matmul(
                                ph, lhsT=wh[:, k, :],
                                rhs=inp_sb[:, k, ts0:ts1],
                                start=(k == 0), stop=(k == KT - 1),
                            )
                        nc.scalar.activation(
                            out=gu_r[:, ts0:ts1], in_=ph, func=SIG,
                            bias=bias_sb[:, e:e + 1],
                        )
                        nc.vector.scalar_tensor_tensor(
                            out=gu_r[:, ts0:ts1], in0=ph,
                            scalar=aux_sb[:, e:e + 1],
                            in1=gu_r[:, ts0:ts1], op0=ADD, op1=MAX,
                        )
                        # u = z*g, then scan this chunk (chained via hst1)
                        nc.vector.tensor_mul(
                            gu_r[:, ts0:ts1], z_r[:, ts0:ts1], gu_r[:, ts0:ts1]
                        )
                        init = (hinit_sb[:, HT + i:HT + i + 1] if t == 0
                                else hst1[:, i:i + 1])
                        nc.vector.tensor_tensor_scan(
                            out=h1_r[:, ts0:ts1], data0=c_r[:, ts0:ts1],
                            data1=gu_r[:, ts0:ts1],
                            initial=init, op0=MULT, op1=ADD,
                        )
                        if t < TC - 1:
                            nc.gpsimd.tensor_copy(
                                hst1[:, i:i + 1], h1_r[:, ts1 - 1:ts1]
                            )
                        else:
                            nc.sync.dma_start(
                                out=hfin.ap()[1, i].unsqueeze(1),
                                in_=h1_r[:, S - 1:S],
                            )
                        # out = h1 + inp (in place), contiguous chunk DMA out
                        nc.gpsimd.tensor_add(
                            h1_r[:, ts0:ts1], h1_r[:, ts0:ts1],
                            inp_sb[:, i, ts0:ts1].bitcast(F32),
                        )
                        nc.sync.dma_start(
                            out=outT.ap()[i, t], in_=h1_r[:, ts0:ts1]
                        )

    nc.compile()
    return nc


def _prepare_shared(w0, b0, w1, b1, h):
    # w*t[e, k, p, m] = W[e*128+m, k*128+p]
    w0t = np.ascontiguousarray(
        w0.T.reshape(KT, 128, ET, 128).transpose(2, 0, 1, 3))
    w1t = np.ascontiguousarray(
        w1.T.reshape(KT, 128, ET, 128).transpose(2, 0, 1, 3))
    # bias[:, l*16 + e] = b_l[e*128 : (e+1)*128]
    bias = np.concatenate(
        [b0.reshape(ET, 128).T, b1.reshape(ET, 128).T], axis=1
    ).astype(np.float32)
    aux0 = np.concatenate([-b0[:H].reshape(HT, 128).T,
                           b0[H:].reshape(HT, 128).T + 0.5], axis=1)
    aux1 = np.concatenate([-b1[:H].reshape(HT, 128).T,
                           b1[H:].reshape(HT, 128).T + 0.5], axis=1)
    aux = np.concatenate([aux0, aux1], axis=1).astype(np.float32)
    return w0t, w1t, bias, np.ascontiguousarray(aux)


def kernel(x, h, w0, b0, w1, b1):
    x = np.asarray(x, np.float32)
    h = np.asarray(h, np.float32)
    w0 = np.asarray(w0, np.float32)
    b0 = np.asarray(b0, np.float32)
    w1 = np.asarray(w1, np.float32)
    b1 = np.asarray(b1, np.float32)

    if "nc" not in _CACHED:
        _CACHED["nc"] = build()
    nc = _CACHED["nc"]

    w0t, w1t, bias, aux = _prepare_shared(w0, b0, w1, b1, h)
    in_maps = []
    for b in range(B):
        # xt[t, k, p, s'] = x[b, t*CS+s', k*128+p]
        xt = np.ascontiguousarray(
            x[b].T.reshape(KT, 128, TC, CS).transpose(2, 0, 1, 3))
        hinit = np.concatenate(
            [h[0, b, 0].reshape(HT, 128).T, h[1, b, 0].reshape(HT, 128).T],
            axis=1,
        ).astype(np.float32)
        in_maps.append({
            "xt": xt, "w0t": w0t, "w1t": w1t,
            "bias": bias, "aux": aux,
            "hinit": np.ascontiguousarray(hinit),
        })

    res = run_bass_kernel_spmd(nc, in_maps, core_ids=list(range(B)))

    out = np.empty((B, S, H), np.float32)
    hfinal = np.empty((2, B, 1, H), np.float32)
    for b in range(B):
        r = res.results[b]
        # outT[i, t, p, s'] -> out[b, t*CS+s', i*128+p]
        out[b] = r["outT"].transpose(0, 2, 1, 3).reshape(H, S).T
        hfinal[:, b, 0, :] = r["hfin"].reshape(2, H)
    return out, hfinal


# revision 16
# speedup vs baseline: 1.3761x; 1.2137x over previous
"""MinGRU (2-layer, residual) Trainium2 Bass kernel.

Problem: B=8, S=2048, D=H=1024, L=2.
Sharding: data-parallel over batch across 8 NeuronCores (1 sample/core);
weights replicated. All tensors device-side are channel-major (channels on
partitions, sequence on the free dimension) so no transposes are needed on
device; the host transposes x / W once and transposes the output back. All
DRAM layouts are pre-tiled host-side so every DMA is fully contiguous.

Per-core pipeline (per layer):
  gh^T = W^T-blocks @ x^T            TensorE, fp32r (full rate, ~1e-4 rel err)
  c = sigmoid(-(gate+b))             ScalarE from PSUM, fused bias/scale
  z = sigmoid(gate+b)                ScalarE
  g = max(hidden+b+0.5, sigmoid(hidden+b))   ScalarE + fused DVE scalar_tensor_tensor
  u = z*g                            DVE
  h_t = c_t*h_{t-1} + u_t            DVE tensor_tensor_scan along S
  residual adds                      GPSIMD

Layer 0 is t-outer: x streams in 512-token chunks, scan chains across chunks
via a small carry buffer; h+x lands in the resident inp buffer (fp32r), which
is layer 1's matmul rhs. Layer 1 is e-outer with gate/hidden interleaved per
chunk: weights stream per 128-channel block (prefetched during layer 0), and
each row's scan chain overlaps its own matmuls, keeping the kernel tail short.
"""
import numpy as np

import concourse.bass as bass
import concourse.mybir as mybir
import concourse.tile as tile
from concourse import bacc
from concourse.bass_utils import run_bass_kernel_spmd

F32 = mybir.dt.float32
F32R = mybir.dt.float32r
SIG = mybir.ActivationFunctionType.Sigmoid
MULT = mybir.AluOpType.mult
ADD = mybir.AluOpType.add
MAX = mybir.AluOpType.max

B, S, D, H = 8, 2048, 1024, 1024
KT = D // 128          # 8  k-tiles (contraction)
HT = H // 128          # 8  h-tiles (per gate/hidden half)
ET = 2 * H // 128      # 16 e-tiles (2H output channels)
TC = 4                 # t-chunks
CS = S // TC           # 512 chunk size (PSUM bank width in fp32)

_CACHED = {}


def build():
    nc = bacc.Bacc(dynamic_dma_scratch_size=2048)

    # all DRAM tensors pre-tiled so each DMA transfer is contiguous
    xt = nc.dram_tensor("xt", [TC, KT, 128, CS], F32R, kind="ExternalInput")
    w0t = nc.dram_tensor("w0t", [HT, 2, 128, KT, 128], F32R, kind="ExternalInput")
    w1t = nc.dram_tensor("w1t", [HT, 2, 128, KT, 128], F32R, kind="ExternalInput")
    bias = nc.dram_tensor("bias", [128, 2 * ET], F32, kind="ExternalInput")
    aux = nc.dram_tensor("aux", [128, 2 * ET], F32, kind="ExternalInput")
    hinit = nc.dram_tensor("hinit", [128, 2 * HT], F32, kind="ExternalInput")

    outT = nc.dram_tensor("outT", [HT, TC, 128, CS], F32, kind="ExternalOutput")
    hfin = nc.dram_tensor("hfin", [2, HT, 128], F32, kind="ExternalOutput")

    with tile.TileContext(nc) as tc:
        with (
            tc.tile_pool(name="singles", bufs=1) as singles,
            tc.tile_pool(name="inp", bufs=1) as inppool,
            tc.tile_pool(name="w1pool", bufs=3) as w1pool,
            tc.tile_pool(name="psum", bufs=8, space="PSUM") as psum_pool,
        ):
            bias_sb = singles.tile([128, 2 * ET], F32)
            aux_sb = singles.tile([128, 2 * ET], F32)
            hinit_sb = singles.tile([128, 2 * HT], F32)
            hst0 = singles.tile([128, HT], F32)
            hst1 = singles.tile([128, HT], F32)
            nc.sync.dma_start(out=bias_sb, in_=bias.ap())
            nc.sync.dma_start(out=aux_sb, in_=aux.ap())
            nc.sync.dma_start(out=hinit_sb, in_=hinit.ap())

            inp_sb = inppool.tile([128, KT, S], F32R, tag="inp")

            # ================= layer 0 (t-outer, streamed x) =================
            with (
                tc.tile_pool(name="wpool", bufs=1) as wpool,
                tc.tile_pool(name="chunks", bufs=2) as chunks,
                tc.tile_pool(name="cz", bufs=1) as czpool,
            ):
                # first x chunk loads BEFORE the weights so the first psum
                # group isn't gated on the full 8MB weight load
                x0 = chunks.tile([128, KT, CS], F32R, tag="chunkbuf")
                for k in range(KT):
                    nc.sync.dma_start(out=x0[:, k, :], in_=xt.ap()[0, k])

                w_sb = wpool.tile([128, KT, 2 * H], F32R, tag="w")
                # weight loads in consumption order (gate i, hidden i+HT
                # pairs); 256KB contiguous source transfers for DMA-queue
                # efficiency (64KB transfers only reach ~90GB/s aggregate)
                for i in range(HT):
                    for g, e in ((0, i), (1, HT + i)):
                        nc.sync.dma_start(
                            out=w_sb[:, :, e * 128:(e + 1) * 128],
                            in_=w0t.ap()[i, g],
                        )

                for t in range(TC):
                    ts0, ts1 = t * CS, (t + 1) * CS
                    if t == 0:
                        x_t = x0
                    else:
                        x_t = chunks.tile([128, KT, CS], F32R, tag="chunkbuf")
                        for k in range(KT):
                            nc.sync.dma_start(out=x_t[:, k, :], in_=xt.ap()[t, k])

                    c_t = czpool.tile([128, HT, CS], F32, tag="c")
                    z_t = czpool.tile([128, HT, CS], F32, tag="z")
                    gu_t = czpool.tile([128, HT, CS], F32, tag="gu")

                    for i in range(HT):
                        # ---- gate e-tile i ----
                        ps = psum_pool.tile([128, CS], F32, tag="ps")
                        for k in range(KT):
                            nc.tensor.matmul(
                                ps,
                                lhsT=w_sb[:, k, i * 128:(i + 1) * 128],
                                rhs=x_t[:, k, :],
                                start=(k == 0),
                                stop=(k == KT - 1),
                            )
                        nc.scalar.activation(
                            out=c_t[:, i, :], in_=ps, func=SIG,
                            bias=aux_sb[:, i:i + 1], scale=-1.0,
                        )
                        nc.scalar.activation(
                            out=z_t[:, i, :], in_=ps, func=SIG,
                            bias=bias_sb[:, i:i + 1],
                        )
                        # ---- hidden e-tile i+HT ----
                        ph = psum_pool.tile([128, CS], F32, tag="ps")
                        e = HT + i
                        for k in range(KT):
                            nc.tensor.matmul(
                                ph,
                                lhsT=w_sb[:, k, e * 128:(e + 1) * 128],
                                rhs=x_t[:, k, :],
                                start=(k == 0),
                                stop=(k == KT - 1),
                            )
                        nc.scalar.activation(
                            out=gu_t[:, i, :], in_=ph, func=SIG,
                            bias=bias_sb[:, e:e + 1],
                        )
                        # g = (hidden + (b+0.5)) max sigmoid(hidden+b)
                        nc.vector.scalar_tensor_tensor(
                            out=gu_t[:, i, :], in0=ph,
                            scalar=aux_sb[:, e:e + 1],
                            in1=gu_t[:, i, :], op0=ADD, op1=MAX,
                        )
                        # u = z * g
                        nc.vector.tensor_mul(gu_t[:, i, :], z_t[:, i, :], gu_t[:, i, :])
                        # ---- scan (chained across chunks via hst0) ----
                        init = (hinit_sb[:, i:i + 1] if t == 0
                                else hst0[:, i:i + 1])
                        nc.vector.tensor_tensor_scan(
                            out=inp_sb[:, i, ts0:ts1],
                            data0=c_t[:, i, :], data1=gu_t[:, i, :],
                            initial=init, op0=MULT, op1=ADD,
                        )
                        # carry pre-residual h for next chunk's initial
                        nc.gpsimd.tensor_copy(
                            hst0[:, i:i + 1], inp_sb[:, i, ts1 - 1:ts1].bitcast(F32)
                        )

                    # residual add, immediately (scan chain carries via hst0)
                    nc.gpsimd.tensor_add(
                        inp_sb[:, :, ts0:ts1], inp_sb[:, :, ts0:ts1], x_t
                    )

                # h0_final = pre-residual h at s = S-1 = the last carry
                for i in range(HT):
                    nc.sync.dma_start(
                        out=hfin.ap()[0, i].unsqueeze(1),
                        in_=hst0[:, i:i + 1],
                    )

            # ==== layer 1 (e-outer, gate/hidden interleaved per chunk) ======
            with tc.tile_pool(name="rows", bufs=2) as rows:
                for i in range(HT):
                    wg = w1pool.tile([128, KT, 128], F32R, tag="w1e")
                    wh = w1pool.tile([128, KT, 128], F32R, tag="w1e")
                    nc.sync.dma_start(out=wg, in_=w1t.ap()[i, 0])
                    nc.sync.dma_start(out=wh, in_=w1t.ap()[i, 1])

                    c_r = rows.tile([128, S], F32, tag="c1")
                    z_r = rows.tile([128, S], F32, tag="z1")
                    gu_r = rows.tile([128, S], F32, tag="gu1")
                    h1_r = rows.tile([128, S], F32, tag="h1")
                    e = ET + HT + i

                    for t in range(TC):
                        ts0, ts1 = t * CS, (t + 1) * CS
                        ps = psum_pool.tile([128, CS], F32, tag="ps")
                        for k in range(KT):
                            nc.tensor.matmul(
                                ps, lhsT=wg[:, k, :],
                                rhs=inp_sb[:, k, ts0:ts1],
                                start=(k == 0), stop=(k == KT - 1),
                            )
                        nc.scalar.activation(
                            out=c_r[:, ts0:ts1], in_=ps, func=SIG,
                            bias=aux_sb[:, ET + i:ET + i + 1], scale=-1.0,
                        )
                        nc.scalar.activation(
                            out=z_r[:, ts0:ts1], in_=ps, func=SIG,
                            bias=bias_sb[:, ET + i:ET + i + 1],
                        )
                        ph = psum_pool.tile([128, CS], F32, tag="ps")
                        for k in range(KT):
                            nc.tensor.matmul(
                                ph, lhsT=wh[:, k, :],
                                rhs=inp_sb[:, k, ts0:ts1],
                                start=(k == 0), stop=(k == KT - 1),
                            )
                        nc.scalar.activation(
                            out=gu_r[:, ts0:ts1], in_=ph, func=SIG,
                            bias=bias_sb[:, e:e + 1],
                        )
                        nc.vector.scalar_tensor_tensor(
                            out=gu_r[:, ts0:ts1], in0=ph,
                            scalar=aux_sb[:, e:e + 1],
                            in1=gu_r[:, ts0:ts1], op0=ADD, op1=MAX,
                        )
                        # u = z*g, then scan this chunk (chained via hst1)
                        nc.vector.tensor_mul(
                            gu_r[:, ts0:ts1], z_r[:, ts0:ts1], gu_r[:, ts0:ts1]
                        )
                        init = (hinit_sb[:, HT + i:HT + i + 1] if t == 0
                                else hst1[:, i:i + 1])
                        nc.vector.tensor_tensor_scan(
                            out=h1_r[:, ts0:ts1], data0=c_r[:, ts0:ts1],
                            data1=gu_r[:, ts0:ts1],
                            initial=init, op0=MULT, op1=ADD,
                        )
                        if t < TC - 1:
                            nc.gpsimd.tensor_copy(
                                hst1[:, i:i + 1], h1_r[:, ts1 - 1:ts1]
                            )
                        else:
                            nc.sync.dma_start(
                                out=hfin.ap()[1, i].unsqueeze(1),
                                in_=h1_r[:, S - 1:S],
                            )
                        # out = h1 + inp (in place), contiguous chunk DMA out
                        nc.gpsimd.tensor_add(
                            h1_r[:, ts0:ts1], h1_r[:, ts0:ts1],
                            inp_sb[:, i, ts0:ts1].bitcast(F32),
                        )
                        nc.sync.dma_start(
                            out=outT.ap()[i, t], in_=h1_r[:, ts0:ts1]
                        )

    nc.compile()
    return nc


def _prepare_shared(w0, b0, w1, b1, h):
    # w*t[i, g, p, k, m] = W[(g*H + i*128)+m, k*128+p]  (gate/hidden pairs,
    # (p,k,m)-ordered so one contiguous 512KB DMA per half-pair)
    def pairs(w):
        blocks = w.T.reshape(KT, 128, ET, 128).transpose(2, 0, 1, 3)
        return np.ascontiguousarray(
            blocks.reshape(2, HT, KT, 128, 128).transpose(1, 0, 3, 2, 4))
    w0t = pairs(w0)
    w1t = pairs(w1)
    # bias[:, l*16 + e] = b_l[e*128 : (e+1)*128]
    bias = np.concatenate(
        [b0.reshape(ET, 128).T, b1.reshape(ET, 128).T], axis=1
    ).astype(np.float32)
    aux0 = np.concatenate([-b0[:H].reshape(HT, 128).T,
                           b0[H:].reshape(HT, 128).T + 0.5], axis=1)
    aux1 = np.concatenate([-b1[:H].reshape(HT, 128).T,
                           b1[H:].reshape(HT, 128).T + 0.5], axis=1)
    aux = np.concatenate([aux0, aux1], axis=1).astype(np.float32)
    return w0t, w1t, bias, np.ascontiguousarray(aux)


def kernel(x, h, w0, b0, w1, b1):
    x = np.asarray(x, np.float32)
    h = np.asarray(h, np.float32)
    w0 = np.asarray(w0, np.float32)
    b0 = np.asarray(b0, np.float32)
    w1 = np.asarray(w1, np.float32)
    b1 = np.asarray(b1, np.float32)

    if "nc" not in _CACHED:
        _CACHED["nc"] = build()
    nc = _CACHED["nc"]

    w0t, w1t, bias, aux = _prepare_shared(w0, b0, w1, b1, h)
    in_maps = []
    for b in range(B):
        # xt[t, k, p, s'] = x[b, t*CS+s', k*128+p]
        xt = np.ascontiguousarray(
            x[b].T.reshape(KT, 128, TC, CS).transpose(2, 0, 1, 3))
        hinit = np.concatenate(
            [h[0, b, 0].reshape(HT, 128).T, h[1, b, 0].reshape(HT, 128).T],
            axis=1,
        ).astype(np.float32)
        in_maps.append({
            "xt": xt, "w0t": w0t, "w1t": w1t,
            "bias": bias, "aux": aux,
            "hinit": np.ascontiguousarray(hinit),
        })

    res = run_bass_kernel_spmd(nc, in_maps, core_ids=list(range(B)))

    out = np.empty((B, S, H), np.float32)
    hfinal = np.empty((2, B, 1, H), np.float32)
    for b in range(B):
        r = res.results[b]
        # outT[i, t, p, s'] -> out[b, t*CS+s', i*128+p]
        out[b] = r["outT"].transpose(0, 2, 1, 3).reshape(H, S).T
        hfinal[:, b, 0, :] = r["hfin"].reshape(2, H)
    return out, hfinal
